# revision 1
# baseline (speedup 1.0000x reference)
"""Trainium2 Bass kernel for Attn_PointLevel (sparse_attention).

Math (per (b,v,p) patch, L=48 tokens, D=512):
  q = Xq @ Wq.T + bq ; k = Xk @ Wkv.T + bkv ; v = Xv @ Wkv.T + bkv
  S = q @ k.T  (48x48), diagonal masked to -inf
  A = softmax(S / sqrt(D)) ;  O = A @ v ;  Y = O @ Wo.T + bo

Kernel strategy (per core = one b-slice, T = 7*24*48 = 8064 tokens):
  - all PE matmuls in bf16 (1 cyc/row at any moving-dim size)
  - feature-major pipeline:
      XqT, XkT via on-chip PE transposes (token-major loads are contiguous)
      qT = WqT-stationary @ XqT   (accumulate 4 d-chunks)
      ST = kT-chunk-stationary @ qT per patch-pair (96 tokens, block-diag)
      ET = exp(scale*ST) * mask01   (mask kills diagonal AND cross-patch terms;
                                     softmax normalization DEFERRED)
      Z  = column-sums of ET via ones-matmul (PE)
      U  = Xv-stationary @ ET      (= (E @ Xv).T, feature-major; Xv never transposed)
      OT = WkvT-stationary @ U     (unnormalized (A@v).T * Z, no bias)
      Y  = OT-chunk-stationary @ WoT  -> token-major PSUM
      Y_sb = Y * (1/Z)[token] + (bo + Wo @ bkv)   (per-partition scale; host-folded bias)
  - biases bq/bkv applied per-partition during qT/kT PSUM->SBUF copy
  - v-projection bias bkv folded into output bias because softmax rows sum to 1.
"""

import numpy as np

B, V, P, L, D = 8, 7, 24, 48, 512
T = V * P * L            # 8064 tokens per core
NCORES = 8
PAIR = 2 * L             # 96 tokens (2 patches) per attention tile
CH = 384                 # tokens per pipeline chunk (3x128 = 4x96)
SCALE = float(D) ** -0.5

_CACHE = {}


def _build(tokens, XINB=3, XTB=2, QKB=2, ATB=2, PSSB=2, PSUB=1, PSYB=2, DVETR=False, OACT=False, MASKMM=True, PSTRB=1, REPS=1, NOTR=False, PREF=True, OUTB=3, PRJB=1, HOIST=True, PSZB=2):
    import concourse.mybir as mybir
    import concourse.tile as tile
    from concourse import bacc

    f32 = mybir.dt.float32
    bf16 = mybir.dt.bfloat16
    AF = mybir.ActivationFunctionType

    nchunks = tokens // CH
    assert tokens % CH == 0

    nc = bacc.Bacc("TRN2", target_bir_lowering=False)

    xq_d = nc.dram_tensor("xq", [tokens, D], bf16, kind="ExternalInput")
    xk_d = nc.dram_tensor("xk", [tokens, D], bf16, kind="ExternalInput")
    xv_d = nc.dram_tensor("xv", [tokens, D], bf16, kind="ExternalInput")
    mt_d = nc.dram_tensor("mt", [D, D], bf16, kind="ExternalInput")
    wvot_d = nc.dram_tensor("wvot", [D, D], bf16, kind="ExternalInput")
    c1_d = nc.dram_tensor("c1cols", [128, 4], f32, kind="ExternalInput")
    bob_d = nc.dram_tensor("bob", [128, D], f32, kind="ExternalInput")
    mask_d = nc.dram_tensor("mask01", [PAIR, PAIR], bf16, kind="ExternalInput")
    ident_d = nc.dram_tensor("ident", [128, 128], bf16, kind="ExternalInput")
    ident96_d = nc.dram_tensor("ident96", [PAIR, PAIR], bf16, kind="ExternalInput")
    ones_d = nc.dram_tensor("ones", [128, 1], bf16, kind="ExternalInput")
    ones1f_d = nc.dram_tensor("ones1f", [1, 1], f32, kind="ExternalInput")
    y_d = nc.dram_tensor("y", [tokens, D], f32, kind="ExternalOutput")

    first_loads = [None]
    with tile.TileContext(nc) as tc:
        with (
            tc.tile_pool(name="const", bufs=1) as constp,
            tc.tile_pool(name="xin", bufs=XINB) as xinp,
            tc.tile_pool(name="xt", bufs=XTB) as xtp,
            tc.tile_pool(name="qkt", bufs=QKB) as qktp,
            tc.tile_pool(name="attn", bufs=ATB) as attnp,
            tc.tile_pool(name="outp", bufs=OUTB) as outp,
            tc.tile_pool(name="ps_tr", bufs=PSTRB, space="PSUM") as ps_tr,
            tc.tile_pool(name="ps_proj", bufs=PRJB, space="PSUM") as ps_proj,
            tc.tile_pool(name="ps_s", bufs=PSSB, space="PSUM") as ps_s,
            tc.tile_pool(name="ps_u", bufs=PSUB, space="PSUM") as ps_u,
            tc.tile_pool(name="ps_zz", bufs=PSZB, space="PSUM") as ps_zz,
            tc.tile_pool(name="ps_y", bufs=PSYB, space="PSUM") as ps_y,
        ):
            def issue_loads(t0):
                # xbar-transposed Xq, Xk loads (feature-major) + Xv pair tiles
                xts = []
                for x_d in (xq_d, xk_d):
                    xt = xtp.tile([128, 4, CH], bf16, tag=f"xt{len(xts)}")
                    for dc in range(4):
                        if NOTR:
                            nc.sync.dma_start(
                                xt[:, dc, :],
                                x_d[t0 : t0 + 128, 0:CH],
                            )
                        else:
                            nc.sync.dma_start_transpose(
                                xt[:, dc, :],
                                x_d[t0 : t0 + CH, dc * 128 : (dc + 1) * 128],
                            )
                    xts.append(xt)
                xv = xinp.tile([PAIR, 4 * CH // PAIR, D], bf16, tag="xv")
                for j in range(CH // PAIR):
                    nc.sync.dma_start(
                        xv[:, j, :], xv_d[t0 + j * PAIR : t0 + (j + 1) * PAIR, :]
                    )
                return xts[0], xts[1], xv

            if HOIST and PREF:
                first_loads[0] = issue_loads(0)

            # persistent constants (issued after the first input loads when
            # HOIST, so chunk-0 DMA-transposes start immediately)
            mt = constp.tile([128, 4, D], bf16, tag="mt")
            wvot = constp.tile([128, 4, D], bf16, tag="wvot")
            nc.sync.dma_start(mt, mt_d[:].rearrange("(o p) e -> p o e", p=128))
            nc.sync.dma_start(wvot, wvot_d[:].rearrange("(o p) e -> p o e", p=128))
            c1 = constp.tile([128, 4], f32, tag="c1")
            bob = constp.tile([128, D], f32, tag="bob")
            nc.sync.dma_start(c1, c1_d[:])
            nc.sync.dma_start(bob, bob_d[:])
            mask = constp.tile([PAIR, PAIR], bf16, tag="mask")
            ident = constp.tile([128, 128], bf16, tag="ident")
            ident96 = constp.tile([PAIR, PAIR], bf16, tag="ident96")
            ones = constp.tile([128, 1], bf16, tag="ones")
            ones1f = constp.tile([1, 1], f32, tag="ones1f")
            nc.sync.dma_start(mask, mask_d[:])
            nc.sync.dma_start(ident, ident_d[:])
            nc.sync.dma_start(ident96, ident96_d[:])
            nc.sync.dma_start(ones, ones_d[:])
            nc.sync.dma_start(ones1f, ones1f_d[:])

            for _rep in range(REPS):
              if PREF:
                  pending = first_loads[0] if (_rep == 0 and HOIST) else issue_loads(0)
              else:
                  pending = None
              for c in range(nchunks):
                t0 = c * CH
                if PREF:
                    xqt, xkt, xv = pending
                    if c + 1 < nchunks:
                        pending = issue_loads((c + 1) * CH)
                else:
                    xqt, xkt, xv = issue_loads(t0)

                # ---- G = M @ XqT + c1 (feature-major); K-projection is folded
                gt = qktp.tile([128, 4, CH], bf16, tag="gt")
                for ec in range(4):
                    psq = ps_proj.tile([128, CH], f32, tag="proj")
                    for dc in range(4):
                        nc.tensor.matmul(
                            psq,
                            mt[:, dc, ec * 128 : (ec + 1) * 128],
                            xqt[:, dc, :],
                            start=(dc == 0),
                            stop=(dc == 3),
                        )
                    nc.scalar.activation(
                        gt[:, ec, :], psq, AF.Identity,
                        bias=c1[:, ec : ec + 1],
                    )

                # ---- attention per pair: ST -> ET -> Z, U
                psz = ps_zz.tile([1, CH], f32, tag="zz")
                u = attnp.tile([128, 4, CH], bf16, tag="u")
                etw = attnp.tile([PAIR, CH], bf16, tag="etw")
                for j in range(CH // PAIR):
                    ls = slice(j * PAIR, (j + 1) * PAIR)
                    pss = ps_s.tile([PAIR, PAIR], f32, tag="s")
                    for ec in range(4):
                        nc.tensor.matmul(
                            pss,
                            xkt[:, ec, ls],
                            gt[:, ec, ls],
                            start=(ec == 0),
                            stop=False if MASKMM else (ec == 3),
                        )
                    if MASKMM:
                        nc.tensor.matmul(
                            pss, mask, ident96, start=False, stop=True
                        )
                    nc.scalar.activation(etw[:, ls], pss, AF.Exp, scale=SCALE)
                    if not MASKMM:
                        nc.vector.tensor_mul(etw[:, ls], etw[:, ls], mask)
                    nc.tensor.matmul(
                        psz[:, ls], ones[:PAIR, :], etw[:, ls], start=True, stop=True
                    )
                    # U[d, l] = sum_m Xv[m, d] * ET[m, l]
                    psu = ps_u.tile([128, 4, PAIR], f32, tag="u")
                    for dc in range(4):
                        nc.tensor.matmul(
                            psu[:, dc, :],
                            xv[:, j, dc * 128 : (dc + 1) * 128],
                            etw[:, ls],
                            start=True,
                            stop=True,
                        )
                    nc.vector.tensor_copy(u[:, :, ls], psu)

                # ---- 1/Z, transposed to per-partition columns per l-tile
                zr = attnp.tile([1, CH], f32, tag="zr")
                nc.vector.tensor_copy(zr, psz)
                zcs = []
                for lt in range(CH // 128):
                    pszc = ps_zz.tile([128, 1], f32, tag="zz")
                    nc.tensor.matmul(
                        pszc,
                        zr[:, lt * 128 : (lt + 1) * 128],
                        ones1f,
                        start=True,
                        stop=True,
                    )
                    zc = attnp.tile([128, 1], f32, tag=f"zc{lt}")
                    nc.vector.reciprocal(zc, pszc)
                    zcs.append(zc)

                # ---- Y = (U.T @ WVO.T) * (1/Z) + bob  (token-major)
                for lt in range(CH // 128):
                    lsl = slice(lt * 128, (lt + 1) * 128)
                    psy = ps_y.tile([128, D], f32, tag="y")
                    for dc in range(4):
                        nc.tensor.matmul(
                            psy,
                            u[:, dc, lsl],
                            wvot[:, dc, :],
                            start=(dc == 0),
                            stop=(dc == 3),
                        )
                    ysb = outp.tile([128, D], f32, tag="ysb")
                    nc.scalar.activation(ysb, psy, AF.Identity, scale=zcs[lt])
                    nc.vector.tensor_add(ysb, ysb, bob)
                    nc.sync.dma_start(y_d[t0 + lt * 128 : t0 + (lt + 1) * 128, :], ysb)

    nc.compile()
    return nc


def _host_inputs(queries, keys, values, Wq, bq, Wkv, bkv, Wo, bo, tokens):
    import ml_dtypes

    bf16 = ml_dtypes.bfloat16
    M = Wkv.astype(np.float64).T @ Wq.astype(np.float64)
    WVO = Wo.astype(np.float64) @ Wkv.astype(np.float64)
    mt = np.ascontiguousarray(M.T).astype(bf16)
    wvot = np.ascontiguousarray(WVO.T).astype(bf16)
    c1v = Wkv.astype(np.float64).T @ bq.astype(np.float64)
    c1cols = np.ascontiguousarray(c1v.reshape(4, 128).T).astype(np.float32)
    bo_eff = (bo.astype(np.float64) + Wo.astype(np.float64) @ bkv.astype(np.float64))
    bob = np.tile(bo_eff.astype(np.float32)[None, :], (128, 1))
    BIG = -1.0e30
    mneg = np.full((PAIR, PAIR), BIG, np.float32)
    blkz = np.eye(L, dtype=np.float32)  # zero where allowed
    mneg[:L, :L] = BIG * blkz
    mneg[L:, L:] = BIG * blkz
    mask01 = mneg.astype(bf16)  # additive -inf mask (diag + cross-patch)
    ident96 = np.eye(PAIR, dtype=np.float32).astype(bf16)
    ident = np.eye(128, dtype=np.float32).astype(bf16)
    ones = np.ones((128, 1), np.float32).astype(bf16)
    ones1f = np.ones((1, 1), np.float32)

    shared = dict(
        mt=mt, wvot=wvot, c1cols=c1cols, bob=bob,
        mask01=mask01, ident=ident, ident96=ident96, ones=ones, ones1f=ones1f,
    )
    in_maps = []
    for core in range(NCORES):
        m = dict(shared)
        m["xq"] = queries[core].reshape(-1, D)[:tokens].astype(bf16)
        m["xk"] = keys[core].reshape(-1, D)[:tokens].astype(bf16)
        m["xv"] = values[core].reshape(-1, D)[:tokens].astype(bf16)
        in_maps.append(m)
    return in_maps


def kernel(queries, keys, values, Wq, bq, Wkv, bkv, Wo, bo, _tokens=T, _trace=False):
    queries = np.asarray(queries)
    keys = np.asarray(keys)
    values = np.asarray(values)
    from concourse.bass_utils import run_bass_kernel_spmd

    key = _tokens
    if key not in _CACHE:
        _CACHE[key] = _build(_tokens)
    nc = _CACHE[key]

    in_maps = _host_inputs(
        queries, keys, values,
        np.asarray(Wq), np.asarray(bq), np.asarray(Wkv), np.asarray(bkv),
        np.asarray(Wo), np.asarray(bo), _tokens,
    )
    res = run_bass_kernel_spmd(
        nc, in_maps, core_ids=list(range(NCORES)), trace=_trace,
    )
    outs = [res.results[i]["y"] for i in range(NCORES)]
    if _tokens == T:
        full = np.stack([o.reshape(V, P, L, D) for o in outs], axis=0)
    else:
        full = np.stack(outs, axis=0)
    if _trace:
        return full, res
    return full



# revision 2
# speedup vs baseline: 1.6041x; 1.6041x over previous
"""Trainium2 Bass kernel for Attn_PointLevel (sparse_attention), v2.

Math (per (b,v,p) patch, L=48 tokens, D=512):
  q = Xq @ Wq.T + bq ; k = Xk @ Wkv.T + bkv ; v = Xv @ Wkv.T + bkv
  S = q @ k.T  (48x48), diagonal masked to -inf
  A = softmax(S / sqrt(D)) ;  O = A @ v ;  Y = O @ Wo.T + bo

Kernel strategy (per core = one b-slice, T = 7*24*48 = 8064 tokens):
  - all PE matmuls in bf16 (1 cyc/row)
  - feature-major pipeline per 384-token chunk:
      XqT, XkT via xbar DMA transposes (HWDGE/SP; only DMAs on HWDGE)
      G  = M @ XqT + c1  (M = Wkv.T Wq folded on host; bias via activation)
      ST = XkT-chunk-stationary @ G per patch-pair (96 tokens, block-diag)
      ET = exp(scale*(ST + mask))   (additive -inf mask folded in via a
                                     mask matmul into PSUM; softmax
                                     normalization DEFERRED)
      Zc = per-token column sums of ET, transposed for free: ones-column
           matmul with ET as stationary -> [tokens,1] PSUM partitions
      U  = Xv-stationary @ ET      (= (E @ Xv).T, feature-major)
      Y  = U.T @ WVO.T -> token-major PSUM  (WVO = Wo @ Wkv folded on host)
      ysb = Y * (1/Zc) + (bo + Wo @ bkv)    (bf16; bias host-folded)
  - xv loads, y stores, and constants go through the Pool engine (SWDGE)
    to keep the single HWDGE issue port free for the 8 input transposes
  - output stored bf16, widened to fp32 on host (exact)
"""

import numpy as np

B, V, P, L, D = 8, 7, 24, 48, 512
T = V * P * L            # 8064 tokens per core
NCORES = 8
PAIR = 2 * L             # 96 tokens (2 patches) per attention tile
CH = 384                 # tokens per pipeline chunk (3x128 = 4x96)
SCALE = float(D) ** -0.5

_CACHE = {}


def _build(tokens, XINB=3, XTB=3, QKB=2, ATB=2, PSSB=2, PSUB=1, PSYB=2,
           PSZB=1, OUTB=3, PRJB=2, MASKMM=0):
    import concourse.mybir as mybir
    import concourse.tile as tile
    from concourse import bacc

    f32 = mybir.dt.float32
    bf16 = mybir.dt.bfloat16
    AF = mybir.ActivationFunctionType

    nchunks = tokens // CH
    assert tokens % CH == 0

    nc = bacc.Bacc("TRN2", target_bir_lowering=False)

    xq_d = nc.dram_tensor("xq", [tokens, D], bf16, kind="ExternalInput")
    xk_d = nc.dram_tensor("xk", [tokens, D], bf16, kind="ExternalInput")
    xv_d = nc.dram_tensor("xv", [tokens, D], bf16, kind="ExternalInput")
    mt_d = nc.dram_tensor("mt", [D, D], bf16, kind="ExternalInput")
    wvot_d = nc.dram_tensor("wvot", [D, D], bf16, kind="ExternalInput")
    c1_d = nc.dram_tensor("c1cols", [128, 4], f32, kind="ExternalInput")
    bob_d = nc.dram_tensor("bob", [128, D], bf16, kind="ExternalInput")
    mask_d = nc.dram_tensor("mask01", [PAIR, PAIR], bf16, kind="ExternalInput")
    ident96_d = nc.dram_tensor("ident96", [PAIR, PAIR], bf16, kind="ExternalInput")
    mask01t_d = nc.dram_tensor("mask01t", [PAIR, PAIR], bf16, kind="ExternalInput")
    ones_d = nc.dram_tensor("ones", [128, 1], bf16, kind="ExternalInput")
    y_d = nc.dram_tensor("y", [tokens, D], bf16, kind="ExternalOutput")

    NT = CH // 128           # 3 output tiles per chunk
    NP = CH // PAIR          # 4 pairs per chunk
    SUP = 3 * CH             # transpose super-chunk (1152 tokens)
    nsup = tokens // SUP
    assert tokens % SUP == 0

    with tile.TileContext(nc) as tc:
        with (
            tc.tile_pool(name="const", bufs=1) as constp,
            tc.tile_pool(name="xin", bufs=XINB) as xinp,
            tc.tile_pool(name="xt", bufs=XTB) as xtp,
            tc.tile_pool(name="qkt", bufs=QKB) as qktp,
            tc.tile_pool(name="attn", bufs=ATB) as attnp,
            tc.tile_pool(name="outp", bufs=OUTB) as outp,
            tc.tile_pool(name="ps_proj", bufs=PRJB, space="PSUM") as ps_proj,
            tc.tile_pool(name="ps_s", bufs=PSSB, space="PSUM") as ps_s,
            tc.tile_pool(name="ps_u", bufs=PSUB, space="PSUM") as ps_u,
            tc.tile_pool(name="ps_zc", bufs=PSZB, space="PSUM") as ps_zc,
            tc.tile_pool(name="ps_y", bufs=PSYB, space="PSUM") as ps_y,
        ):
            # early constants on Pool (SWDGE): needed by the first G matmuls
            mt = constp.tile([128, 4, D], bf16, tag="mt")
            nc.gpsimd.dma_start(mt, mt_d[:].rearrange("(o p) e -> p o e", p=128))
            c1 = constp.tile([128, 4], f32, tag="c1")
            nc.gpsimd.dma_start(c1, c1_d[:])

            def issue_transposes(s):
                # xbar-transposed Xq, Xk (feature-major) for a whole
                # super-chunk (3 compute chunks) in 8 HWDGE DMAs on SP
                t0 = s * SUP
                xts = []
                for x_d in (xq_d, xk_d):
                    xt = xtp.tile([128, 4, SUP], bf16, tag=f"xt{len(xts)}")
                    for dc in range(4):
                        nc.sync.dma_start_transpose(
                            xt[:, dc, :],
                            x_d[t0 : t0 + SUP, dc * 128 : (dc + 1) * 128],
                        )
                    xts.append(xt)
                return xts

            def issue_xv(s, split=False):
                # Xv pair tiles for a whole super-chunk in ONE SWDGE dma
                # (split per chunk for super 0 so chunk 0's slice lands early)
                xv = xinp.tile([PAIR, 3 * NP, D], bf16, tag="xv")
                pieces = range(3) if split else (slice(None),)
                for i in pieces:
                    isl = slice(i, i + 1) if isinstance(i, int) else i
                    t0 = s * SUP + (0 if not isinstance(i, int) else i * CH)
                    n = CH if isinstance(i, int) else SUP
                    nc.gpsimd.dma_start(
                        xv[:, isl.start * NP : isl.start * NP + NP, :]
                        if isinstance(i, int)
                        else xv,
                        xv_d[t0 : t0 + n, :].rearrange("(j p) d -> p j d", p=PAIR),
                    )
                return xv

            pend_t = [issue_transposes(0)]
            pend_v = [issue_xv(0)]

            # remaining constants (needed mid-chunk-0 or later)
            ones = constp.tile([128, 1], bf16, tag="ones")
            nc.gpsimd.dma_start(ones, ones_d[:])
            if MASKMM:
                mask = constp.tile([PAIR, PAIR], bf16, tag="mask")
                ident96 = constp.tile([PAIR, PAIR], bf16, tag="ident96")
                nc.gpsimd.dma_start(mask, mask_d[:])
                nc.gpsimd.dma_start(ident96, ident96_d[:])
            else:
                mask01t = constp.tile([PAIR, PAIR], bf16, tag="mask01t")
                nc.gpsimd.dma_start(mask01t, mask01t_d[:])
            wvot = constp.tile([128, 4, D], bf16, tag="wvot")
            bob = constp.tile([128, D], bf16, tag="bob")
            nc.gpsimd.dma_start(wvot, wvot_d[:].rearrange("(o p) e -> p o e", p=128))
            nc.gpsimd.dma_start(bob, bob_d[:])

            for c in range(nchunks):
                t0 = c * CH
                if c % 3 == 0:
                    xqt_s, xkt_s = pend_t.pop(0)
                    xv_s = pend_v.pop(0)
                    ysb_s = outp.tile([128, 3 * NT, D], bf16, tag="ysb")
                if c % 3 == 1 and c + 2 < nchunks:
                    pend_t.append(issue_transposes(c // 3 + 1))
                if c % 3 == 2 and c + 1 < nchunks:
                    pend_v.append(issue_xv(c // 3 + 1))
                off = (c % 3) * CH
                xqt = xqt_s[:, :, off : off + CH]
                xkt = xkt_s[:, :, off : off + CH]

                # ---- G = M @ XqT + c1 (feature-major)
                gt = qktp.tile([128, 4, CH], bf16, tag="gt")
                for ec in range(4):
                    psq = ps_proj.tile([128, CH], f32, tag="proj")
                    for dc in range(4):
                        nc.tensor.matmul(
                            psq,
                            mt[:, dc, ec * 128 : (ec + 1) * 128],
                            xqt[:, dc, :],
                            start=(dc == 0),
                            stop=(dc == 3),
                        )
                    nc.scalar.activation(
                        gt[:, ec, :], psq, AF.Identity,
                        bias=c1[:, ec : ec + 1],
                    )

                # ---- attention per pair: ST -> ET -> Zc, U
                pszc = ps_zc.tile([128, NT], f32, tag="zc")
                u = attnp.tile([128, 4, CH], bf16, tag="u")
                etw = attnp.tile([PAIR, CH], bf16, tag="etw")
                for j in range(NP):
                    ls = slice(j * PAIR, (j + 1) * PAIR)
                    pss = ps_s.tile([PAIR, PAIR], f32, tag="s")
                    for ec in range(4):
                        nc.tensor.matmul(
                            pss,
                            xkt[:, ec, ls],
                            gt[:, ec, ls],
                            start=(ec == 0),
                            stop=False,
                        )
                    if MASKMM:
                        nc.tensor.matmul(pss, mask, ident96, start=False, stop=True)
                        nc.scalar.activation(etw[:, ls], pss, AF.Exp, scale=SCALE)
                    else:
                        nc.scalar.activation(etw[:, ls], pss, AF.Exp, scale=SCALE)
                        nc.vector.tensor_mul(etw[:, ls], etw[:, ls], mask01t)
                    # U[d, l] = sum_m Xv[m, d] * ET[m, l]
                    psu = ps_u.tile([128, 4, PAIR], f32, tag="u")
                    for dc in range(4):
                        nc.tensor.matmul(
                            psu[:, dc, :],
                            xv_s[:, (c % 3) * NP + j, dc * 128 : (dc + 1) * 128],
                            etw[:, ls],
                            start=True,
                            stop=True,
                        )
                    nc.vector.tensor_copy(u[:, :, ls], psu)

                # ---- Zc[token,1] per 128-token tile: ET-stationary @ ones
                # gives the transposed column sums directly (free size 1 on PE)
                for lt in range(NT):
                    nc.tensor.matmul(
                        pszc[:, lt : lt + 1],
                        etw[:, lt * 128 : (lt + 1) * 128],
                        ones[:PAIR, :],
                        start=True,
                        stop=True,
                    )
                zcs = attnp.tile([128, NT], f32, tag="zcs")
                nc.vector.reciprocal(zcs, pszc)

                # ---- Y = (U.T @ WVO.T) * (1/Zc) + bob  (token-major, bf16)
                # fused scale+bias on DVE keeps Activation free for gt copies
                for lt in range(NT):
                    lsl = slice(lt * 128, (lt + 1) * 128)
                    psy = ps_y.tile([128, D], f32, tag="y")
                    for dc in range(4):
                        nc.tensor.matmul(
                            psy,
                            u[:, dc, lsl],
                            wvot[:, dc, :],
                            start=(dc == 0),
                            stop=(dc == 3),
                        )
                    nc.vector.scalar_tensor_tensor(
                        ysb_s[:, (c % 3) * NT + lt, :], psy, zcs[:, lt : lt + 1],
                        bob,
                        op0=mybir.AluOpType.mult, op1=mybir.AluOpType.add,
                    )
                last_super = c >= nchunks - 3
                if last_super:
                    # per-chunk stores in the final super shorten the drain
                    nc.gpsimd.dma_start(
                        y_d[t0 : t0 + CH, :].rearrange("(t p) e -> p t e", p=128),
                        ysb_s[:, (c % 3) * NT : (c % 3 + 1) * NT, :],
                    )
                elif c % 3 == 2:
                    # one SWDGE store per super-chunk on Pool
                    nc.gpsimd.dma_start(
                        y_d[t0 + CH - SUP : t0 + CH, :].rearrange(
                            "(t p) e -> p t e", p=128
                        ),
                        ysb_s,
                    )

    nc.compile()
    return nc


def _host_inputs(queries, keys, values, Wq, bq, Wkv, bkv, Wo, bo, tokens):
    import ml_dtypes

    bf16 = ml_dtypes.bfloat16
    M = Wkv.astype(np.float64).T @ Wq.astype(np.float64)
    WVO = Wo.astype(np.float64) @ Wkv.astype(np.float64)
    mt = np.ascontiguousarray(M.T).astype(bf16)
    wvot = np.ascontiguousarray(WVO.T).astype(bf16)
    c1v = Wkv.astype(np.float64).T @ bq.astype(np.float64)
    c1cols = np.ascontiguousarray(c1v.reshape(4, 128).T).astype(np.float32)
    bo_eff = (bo.astype(np.float64) + Wo.astype(np.float64) @ bkv.astype(np.float64))
    bob = np.tile(bo_eff.astype(np.float32)[None, :], (128, 1)).astype(bf16)
    BIG = -1.0e30
    mneg = np.full((PAIR, PAIR), BIG, np.float32)
    blkz = np.eye(L, dtype=np.float32)  # zero where allowed
    mneg[:L, :L] = BIG * blkz
    mneg[L:, L:] = BIG * blkz
    mask01 = mneg.astype(bf16)  # additive -inf mask (diag + cross-patch)
    ident96 = np.eye(PAIR, dtype=np.float32).astype(bf16)
    ones = np.ones((128, 1), np.float32).astype(bf16)
    m01 = np.ones((PAIR, PAIR), np.float32)
    m01[:L, :L] -= np.eye(L)
    m01[L:, L:] -= np.eye(L)
    m01[:L, L:] = 0.0
    m01[L:, :L] = 0.0
    mask01t = m01.astype(bf16)

    shared = dict(
        mt=mt, wvot=wvot, c1cols=c1cols, bob=bob,
        mask01=mask01, ident96=ident96, ones=ones, mask01t=mask01t,
    )
    in_maps = []
    for core in range(NCORES):
        m = dict(shared)
        m["xq"] = queries[core].reshape(-1, D)[:tokens].astype(bf16)
        m["xk"] = keys[core].reshape(-1, D)[:tokens].astype(bf16)
        m["xv"] = values[core].reshape(-1, D)[:tokens].astype(bf16)
        in_maps.append(m)
    return in_maps


def kernel(queries, keys, values, Wq, bq, Wkv, bkv, Wo, bo, _tokens=T, _trace=False):
    queries = np.asarray(queries)
    keys = np.asarray(keys)
    values = np.asarray(values)
    from concourse.bass_utils import run_bass_kernel_spmd

    key = _tokens
    if key not in _CACHE:
        _CACHE[key] = _build(_tokens)
    nc = _CACHE[key]

    in_maps = _host_inputs(
        queries, keys, values,
        np.asarray(Wq), np.asarray(bq), np.asarray(Wkv), np.asarray(bkv),
        np.asarray(Wo), np.asarray(bo), _tokens,
    )
    res = run_bass_kernel_spmd(
        nc, in_maps, core_ids=list(range(NCORES)), trace=_trace,
    )
    outs = [np.asarray(res.results[i]["y"]).astype(np.float32) for i in range(NCORES)]
    if _tokens == T:
        full = np.stack([o.reshape(V, P, L, D) for o in outs], axis=0)
    else:
        full = np.stack(outs, axis=0)
    if _trace:
        return full, res
    return full


# revision 4
# speedup vs baseline: 1.8141x; 1.1309x over previous
"""Trainium2 Bass kernel for Attn_PointLevel (sparse_attention), v2.

Math (per (b,v,p) patch, L=48 tokens, D=512):
  q = Xq @ Wq.T + bq ; k = Xk @ Wkv.T + bkv ; v = Xv @ Wkv.T + bkv
  S = q @ k.T  (48x48), diagonal masked to -inf
  A = softmax(S / sqrt(D)) ;  O = A @ v ;  Y = O @ Wo.T + bo

Kernel strategy (per core = one b-slice, T = 7*24*48 = 8064 tokens):
  - all PE matmuls in bf16 (1 cyc/row)
  - feature-major pipeline per 384-token chunk:
      XqT, XkT via xbar DMA transposes (HWDGE/SP; only DMAs on HWDGE)
      G  = M @ XqT + c1  (M = Wkv.T Wq folded on host; bias via activation)
      ST = XkT-chunk-stationary @ G per patch-pair (96 tokens, block-diag)
      ET = exp(scale*(ST + mask))   (additive -inf mask folded in via a
                                     mask matmul into PSUM; softmax
                                     normalization DEFERRED)
      Zc = per-token column sums of ET, transposed for free: ones-column
           matmul with ET as stationary -> [tokens,1] PSUM partitions
      U  = Xv-stationary @ ET      (= (E @ Xv).T, feature-major)
      Y  = U.T @ WVO.T -> token-major PSUM  (WVO = Wo @ Wkv folded on host)
      ysb = Y * (1/Zc) + (bo + Wo @ bkv)    (bf16; bias host-folded)
  - xv loads, y stores, and constants go through the Pool engine (SWDGE)
    to keep the single HWDGE issue port free for the 8 input transposes
  - output stored bf16, widened to fp32 on host (exact)
"""

import numpy as np

B, V, P, L, D = 8, 7, 24, 48, 512
T = V * P * L            # 8064 tokens per core
NCORES = 8
PAIR = 2 * L             # 96 tokens (2 patches) per attention tile
CH = 384                 # tokens per pipeline chunk (3x128 = 4x96)
SCALE = float(D) ** -0.5

_CACHE = {}


def _build(tokens, XINB=3, XTB=3, QKB=2, ATB=2, PSSB=2, PSUB=1, PSYB=2,
           PSZB=1, OUTB=3, PRJB=2):
    import concourse.mybir as mybir
    import concourse.tile as tile
    from concourse import bacc

    f32 = mybir.dt.float32
    bf16 = mybir.dt.bfloat16
    AF = mybir.ActivationFunctionType

    nchunks = tokens // CH
    assert tokens % CH == 0

    nc = bacc.Bacc("TRN2", target_bir_lowering=False)

    xq_d = nc.dram_tensor("xq", [tokens, D], bf16, kind="ExternalInput")
    xk_d = nc.dram_tensor("xk", [tokens, D], bf16, kind="ExternalInput")
    xv_d = nc.dram_tensor("xv", [tokens, D], bf16, kind="ExternalInput")
    CW = 4 * D + 4 * D + D + PAIR + 1 + 4    # packed const width (4709)
    cst_d = nc.dram_tensor("cst", [128, CW], bf16, kind="ExternalInput")
    y_d = nc.dram_tensor("y", [tokens, D], bf16, kind="ExternalOutput")

    NT = CH // 128           # 3 output tiles per chunk
    NP = CH // PAIR          # 4 pairs per chunk
    SUP = 3 * CH             # xv/ysb super-chunk (1152 tokens)
    TSUP = 7 * CH            # transpose super-chunk (2688 tokens)
    nsup = tokens // SUP
    assert tokens % SUP == 0 and tokens % TSUP == 0

    with tile.TileContext(nc) as tc:
        with (
            tc.tile_pool(name="const", bufs=1) as constp,
            tc.tile_pool(name="xin", bufs=XINB) as xinp,
            tc.tile_pool(name="xt", bufs=XTB) as xtp,
            tc.tile_pool(name="qkt", bufs=QKB) as qktp,
            tc.tile_pool(name="attn", bufs=ATB) as attnp,
            tc.tile_pool(name="outp", bufs=OUTB) as outp,
            tc.tile_pool(name="ps_proj", bufs=PRJB, space="PSUM") as ps_proj,
            tc.tile_pool(name="ps_s", bufs=PSSB, space="PSUM") as ps_s,
            tc.tile_pool(name="ps_u", bufs=PSUB, space="PSUM") as ps_u,
            tc.tile_pool(name="ps_zc", bufs=PSZB, space="PSUM") as ps_zc,
            tc.tile_pool(name="ps_y", bufs=PSYB, space="PSUM") as ps_y,
        ):
            # ALL constants in ONE DMA on the Act HWDGE queue: fewest links
            # in the sem-chain (emitted after the first transpose batch so
            # the xq transposes head the startup DMA chain)
            cst = constp.tile([128, CW], bf16, tag="cst")

            def emit_cst():
                nc.scalar.dma_start(cst, cst_d[:])

            CA = 4 * D + PAIR + 5    # critical prefix width

            def mt(dc, e0, e1):
                return cst[:, dc * D + e0 : dc * D + e1]

            mask01t = cst[:PAIR, 4 * D : 4 * D + PAIR]
            onesc = cst[:PAIR, 4 * D + PAIR : 4 * D + PAIR + 1]

            def c1(ec):
                return cst[:, 4 * D + PAIR + 1 + ec : 4 * D + PAIR + 2 + ec]

            def wvot(dc):
                return cst[:, CA + dc * D : CA + (dc + 1) * D]

            bob = cst[:, CA + 4 * D : CA + 5 * D]

            def issue_transposes(s, split=False):
                # xbar-transposed Xq, Xk (feature-major) for a whole
                # super-chunk (3 compute chunks) in 8 HWDGE DMAs on SP.
                # split=True (startup): chunk-0 slices first so compute can
                # begin while the rest of the super-chunk transposes land.
                t0 = s * SUP
                xts = []
                for x_d in (xq_d, xk_d):
                    xt = xtp.tile([128, 4, SUP], bf16, tag=f"xt{len(xts)}")
                    for dc in range(4):
                        nc.sync.dma_start_transpose(
                            xt[:, dc, :],
                            x_d[t0 : t0 + SUP, dc * 128 : (dc + 1) * 128],
                        )
                    xts.append(xt)
                return xts

            def issue_xv(s, engine=None):
                # Xv pair tiles for a whole super-chunk in ONE SWDGE dma
                xv = xinp.tile([PAIR, 3 * NP, D], bf16, tag="xv")
                (engine or nc.gpsimd).dma_start(
                    xv,
                    xv_d[s * SUP : (s + 1) * SUP, :].rearrange(
                        "(j p) d -> p j d", p=PAIR
                    ),
                )
                return xv

            pend_t = [issue_transposes(0)]
            emit_cst()
            pend_v = [issue_xv(0, engine=nc.scalar)]

            # remaining constants (needed mid-chunk-0 or later)

            for c in range(nchunks):
                t0 = c * CH
                if c % 3 == 0:
                    xqt_s, xkt_s = pend_t.pop(0)
                    xv_s = pend_v.pop(0)
                    ysb_s = outp.tile([128, 3 * NT, D], bf16, tag="ysb")
                if c % 3 == 1 and c + 2 < nchunks:
                    pend_t.append(issue_transposes(c // 3 + 1))
                if c % 3 == 2 and c + 1 < nchunks:
                    pend_v.append(issue_xv(c // 3 + 1))
                off = (c % 3) * CH
                xqt = xqt_s[:, :, off : off + CH]
                xkt = xkt_s[:, :, off : off + CH]

                # ---- G = M @ XqT + c1 (feature-major)
                gt = qktp.tile([128, 4, CH], bf16, tag="gt")
                for ec in range(4):
                    psq = ps_proj.tile([128, CH], f32, tag="proj")
                    for dc in range(4):
                        nc.tensor.matmul(
                            psq,
                            mt(dc, ec * 128, (ec + 1) * 128),
                            xqt[:, dc, :],
                            start=(dc == 0),
                            stop=(dc == 3),
                        )
                    nc.scalar.activation(
                        gt[:, ec, :], psq, AF.Identity,
                        bias=c1(ec),
                    )

                # ---- attention per pair: ST -> ET -> Zc, U
                pszc = ps_zc.tile([128, NT], f32, tag="zc")
                u = attnp.tile([128, 4, CH], bf16, tag="u")
                etw = attnp.tile([PAIR, CH], bf16, tag="etw")
                for j in range(NP):
                    ls = slice(j * PAIR, (j + 1) * PAIR)
                    pss = ps_s.tile([PAIR, PAIR], f32, tag="s")
                    for ec in range(4):
                        nc.tensor.matmul(
                            pss,
                            xkt[:, ec, ls],
                            gt[:, ec, ls],
                            start=(ec == 0),
                            stop=False,
                        )
                    nc.scalar.activation(etw[:, ls], pss, AF.Exp, scale=SCALE)
                    nc.vector.tensor_mul(etw[:, ls], etw[:, ls], mask01t)
                    # U[d, l] = sum_m Xv[m, d] * ET[m, l]
                    psu = ps_u.tile([128, 4, PAIR], f32, tag="u")
                    for dc in range(4):
                        nc.tensor.matmul(
                            psu[:, dc, :],
                            xv_s[:, (c % 3) * NP + j, dc * 128 : (dc + 1) * 128],
                            etw[:, ls],
                            start=True,
                            stop=True,
                        )
                    nc.vector.tensor_copy(u[:, :, ls], psu)

                # ---- Zc[token,1] per 128-token tile: ET-stationary @ ones
                # gives the transposed column sums directly (free size 1 on PE)
                for lt in range(NT):
                    nc.tensor.matmul(
                        pszc[:, lt : lt + 1],
                        etw[:, lt * 128 : (lt + 1) * 128],
                        onesc,
                        start=True,
                        stop=True,
                    )
                zcs = attnp.tile([128, NT], f32, tag="zcs")
                nc.vector.reciprocal(zcs, pszc)

                # ---- Y = (U.T @ WVO.T) * (1/Zc) + bob  (token-major, bf16)
                # fused scale+bias on DVE keeps Activation free for gt copies
                for lt in range(NT):
                    lsl = slice(lt * 128, (lt + 1) * 128)
                    psy = ps_y.tile([128, D], f32, tag="y")
                    for dc in range(4):
                        nc.tensor.matmul(
                            psy,
                            u[:, dc, lsl],
                            wvot(dc),
                            start=(dc == 0),
                            stop=(dc == 3),
                        )
                    nc.vector.scalar_tensor_tensor(
                        ysb_s[:, (c % 3) * NT + lt, :], psy, zcs[:, lt : lt + 1],
                        bob,
                        op0=mybir.AluOpType.mult, op1=mybir.AluOpType.add,
                    )
                last_super = c >= nchunks - 3
                if c == nchunks - 1:
                    # final chunk: store each 128-token tile as soon as ready
                    for lt2 in range(NT):
                        nc.gpsimd.dma_start(
                            y_d[t0 + lt2 * 128 : t0 + (lt2 + 1) * 128, :].rearrange(
                                "(t p) e -> p t e", p=128
                            ),
                            ysb_s[:, (c % 3) * NT + lt2 : (c % 3) * NT + lt2 + 1, :],
                        )
                elif last_super:
                    # per-chunk stores in the final super shorten the drain
                    nc.gpsimd.dma_start(
                        y_d[t0 : t0 + CH, :].rearrange("(t p) e -> p t e", p=128),
                        ysb_s[:, (c % 3) * NT : (c % 3 + 1) * NT, :],
                    )
                elif c % 3 == 2:
                    # one SWDGE store per super-chunk on Pool
                    nc.gpsimd.dma_start(
                        y_d[t0 + CH - SUP : t0 + CH, :].rearrange(
                            "(t p) e -> p t e", p=128
                        ),
                        ysb_s,
                    )

    nc.compile()
    return nc


def _host_inputs(queries, keys, values, Wq, bq, Wkv, bkv, Wo, bo, tokens):
    import ml_dtypes

    bf16 = ml_dtypes.bfloat16
    M = Wkv.astype(np.float64).T @ Wq.astype(np.float64)
    WVO = Wo.astype(np.float64) @ Wkv.astype(np.float64)
    # packed const block [128, 4D+4D+D+PAIR+1+4] (see _build)
    CW = 9 * D + PAIR + 5
    CA = 4 * D + PAIR + 5
    cst = np.zeros((128, CW), np.float32)
    cst[:, 0 : 4 * D] = M.T.reshape(4, 128, D).transpose(1, 0, 2).reshape(128, 4 * D)
    m01 = np.ones((PAIR, PAIR), np.float32)
    m01[:L, :L] -= np.eye(L)
    m01[L:, L:] -= np.eye(L)
    m01[:L, L:] = 0.0
    m01[L:, :L] = 0.0
    cst[:PAIR, 4 * D : 4 * D + PAIR] = m01
    cst[:, 4 * D + PAIR] = 1.0
    c1v = Wkv.astype(np.float64).T @ bq.astype(np.float64)
    cst[:, 4 * D + PAIR + 1 : CA] = c1v.reshape(4, 128).T
    cst[:, CA : CA + 4 * D] = (
        WVO.T.reshape(4, 128, D).transpose(1, 0, 2).reshape(128, 4 * D)
    )
    bo_eff = (bo.astype(np.float64) + Wo.astype(np.float64) @ bkv.astype(np.float64))
    cst[:, CA + 4 * D :] = bo_eff[None, :]
    shared = dict(cst=cst.astype(bf16))
    in_maps = []
    for core in range(NCORES):
        m = dict(shared)
        m["xq"] = queries[core].reshape(-1, D)[:tokens].astype(bf16)
        m["xk"] = keys[core].reshape(-1, D)[:tokens].astype(bf16)
        m["xv"] = values[core].reshape(-1, D)[:tokens].astype(bf16)
        in_maps.append(m)
    return in_maps


def kernel(queries, keys, values, Wq, bq, Wkv, bkv, Wo, bo, _tokens=T, _trace=False):
    queries = np.asarray(queries)
    keys = np.asarray(keys)
    values = np.asarray(values)
    from concourse.bass_utils import run_bass_kernel_spmd

    key = _tokens
    if key not in _CACHE:
        _CACHE[key] = _build(_tokens)
    nc = _CACHE[key]

    in_maps = _host_inputs(
        queries, keys, values,
        np.asarray(Wq), np.asarray(bq), np.asarray(Wkv), np.asarray(bkv),
        np.asarray(Wo), np.asarray(bo), _tokens,
    )
    res = run_bass_kernel_spmd(
        nc, in_maps, core_ids=list(range(NCORES)), trace=_trace,
    )
    outs = [np.asarray(res.results[i]["y"]).astype(np.float32) for i in range(NCORES)]
    if _tokens == T:
        full = np.stack([o.reshape(V, P, L, D) for o in outs], axis=0)
    else:
        full = np.stack(outs, axis=0)
    if _trace:
        return full, res
    return full


# revision 5
# speedup vs baseline: 1.8383x; 1.0134x over previous
"""Trainium2 Bass kernel for Attn_PointLevel (sparse_attention), v2.

Math (per (b,v,p) patch, L=48 tokens, D=512):
  q = Xq @ Wq.T + bq ; k = Xk @ Wkv.T + bkv ; v = Xv @ Wkv.T + bkv
  S = q @ k.T  (48x48), diagonal masked to -inf
  A = softmax(S / sqrt(D)) ;  O = A @ v ;  Y = O @ Wo.T + bo

Kernel strategy (per core = one b-slice, T = 7*24*48 = 8064 tokens):
  - all PE matmuls in bf16 (1 cyc/row)
  - feature-major pipeline per 384-token chunk:
      XqT, XkT via xbar DMA transposes (HWDGE/SP; only DMAs on HWDGE)
      G  = M @ XqT + c1  (M = Wkv.T Wq folded on host; bias via activation)
      ST = XkT-chunk-stationary @ G per patch-pair (96 tokens, block-diag)
      ET = exp(scale*(ST + mask))   (additive -inf mask folded in via a
                                     mask matmul into PSUM; softmax
                                     normalization DEFERRED)
      Zc = per-token column sums of ET, transposed for free: ones-column
           matmul with ET as stationary -> [tokens,1] PSUM partitions
      U  = Xv-stationary @ ET      (= (E @ Xv).T, feature-major)
      Y  = U.T @ WVO.T -> token-major PSUM  (WVO = Wo @ Wkv folded on host)
      ysb = Y * (1/Zc) + (bo + Wo @ bkv)    (bf16; bias host-folded)
  - xv loads, y stores, and constants go through the Pool engine (SWDGE)
    to keep the single HWDGE issue port free for the 8 input transposes
  - output stored bf16, widened to fp32 on host (exact)
"""

import numpy as np

B, V, P, L, D = 8, 7, 24, 48, 512
T = V * P * L            # 8064 tokens per core
NCORES = 8
PAIR = 2 * L             # 96 tokens (2 patches) per attention tile
CH = 384                 # tokens per pipeline chunk (3x128 = 4x96)
SUPH = 3 * CH            # super-chunk for host-side xq/xk interleaving
SCALE = float(D) ** -0.5

_CACHE = {}


def _build(tokens, XINB=3, XTB=3, QKB=2, ATB=2, PSSB=2, PSUB=1, PSYB=2,
           PSZB=1, OUTB=3, PRJB=2):
    import concourse.mybir as mybir
    import concourse.tile as tile
    from concourse import bacc

    f32 = mybir.dt.float32
    bf16 = mybir.dt.bfloat16
    AF = mybir.ActivationFunctionType

    nchunks = tokens // CH
    assert tokens % CH == 0

    nc = bacc.Bacc("TRN2", target_bir_lowering=False)

    # xq and xk interleaved per super-chunk: [s][q|k][SUP, D] — lets one
    # xbar-transpose DMA per d-chunk move both tensors at once
    xqk_d = nc.dram_tensor("xqk", [2 * tokens, D], bf16, kind="ExternalInput")
    xv_d = nc.dram_tensor("xv", [tokens, D], bf16, kind="ExternalInput")
    CW = 4 * D + 4 * D + D + PAIR + 1 + 4    # packed const width (4709)
    cst_d = nc.dram_tensor("cst", [128, CW], bf16, kind="ExternalInput")
    y_d = nc.dram_tensor("y", [tokens, D], bf16, kind="ExternalOutput")

    NT = CH // 128           # 3 output tiles per chunk
    NP = CH // PAIR          # 4 pairs per chunk
    SUP = 3 * CH             # xv/ysb super-chunk (1152 tokens)
    TSUP = 7 * CH            # transpose super-chunk (2688 tokens)
    nsup = tokens // SUP
    assert tokens % SUP == 0 and tokens % TSUP == 0

    with tile.TileContext(nc) as tc:
        with (
            tc.tile_pool(name="const", bufs=1) as constp,
            tc.tile_pool(name="xin", bufs=XINB) as xinp,
            tc.tile_pool(name="xt", bufs=XTB) as xtp,
            tc.tile_pool(name="qkt", bufs=QKB) as qktp,
            tc.tile_pool(name="attn", bufs=ATB) as attnp,
            tc.tile_pool(name="outp", bufs=OUTB) as outp,
            tc.tile_pool(name="ps_proj", bufs=PRJB, space="PSUM") as ps_proj,
            tc.tile_pool(name="ps_s", bufs=PSSB, space="PSUM") as ps_s,
            tc.tile_pool(name="ps_u", bufs=PSUB, space="PSUM") as ps_u,
            tc.tile_pool(name="ps_zc", bufs=PSZB, space="PSUM") as ps_zc,
            tc.tile_pool(name="ps_y", bufs=PSYB, space="PSUM") as ps_y,
        ):
            # ALL constants in ONE DMA on the Act HWDGE queue: fewest links
            # in the sem-chain (emitted after the first transpose batch so
            # the xq transposes head the startup DMA chain)
            cst = constp.tile([128, CW], bf16, tag="cst")

            def emit_cst():
                nc.scalar.dma_start(cst, cst_d[:])

            CA = 4 * D + PAIR + 5    # critical prefix width

            def mt(dc, e0, e1):
                return cst[:, dc * D + e0 : dc * D + e1]

            mask01t = cst[:PAIR, 4 * D : 4 * D + PAIR]
            onesc = cst[:PAIR, 4 * D + PAIR : 4 * D + PAIR + 1]

            def c1(ec):
                return cst[:, 4 * D + PAIR + 1 + ec : 4 * D + PAIR + 2 + ec]

            def wvot(dc):
                return cst[:, CA + dc * D : CA + (dc + 1) * D]

            bob = cst[:, CA + 4 * D : CA + 5 * D]

            def issue_transposes(s):
                # xbar-transposed Xq+Xk (feature-major) for a whole
                # super-chunk (3 compute chunks) in 4 HWDGE DMAs on SP
                t0 = s * 2 * SUP
                xt = xtp.tile([128, 4, 2 * SUP], bf16, tag="xt")
                for dc in range(4):
                    nc.sync.dma_start_transpose(
                        xt[:, dc, :],
                        xqk_d[t0 : t0 + 2 * SUP, dc * 128 : (dc + 1) * 128],
                    )
                return xt

            def issue_xv(s, engine=None):
                # Xv pair tiles for a whole super-chunk in ONE SWDGE dma
                xv = xinp.tile([PAIR, 3 * NP, D], bf16, tag="xv")
                (engine or nc.gpsimd).dma_start(
                    xv,
                    xv_d[s * SUP : (s + 1) * SUP, :].rearrange(
                        "(j p) d -> p j d", p=PAIR
                    ),
                )
                return xv

            pend_t = [issue_transposes(0)]
            emit_cst()
            pend_v = [issue_xv(0, engine=nc.scalar)]

            # remaining constants (needed mid-chunk-0 or later)

            for c in range(nchunks):
                t0 = c * CH
                if c % 3 == 0:
                    xt_s = pend_t.pop(0)
                    xv_s = pend_v.pop(0)
                    ysb_s = outp.tile([128, 3 * NT, D], bf16, tag="ysb")
                if c % 3 == 1 and c + 2 < nchunks:
                    pend_t.append(issue_transposes(c // 3 + 1))
                if c % 3 == 2 and c + 1 < nchunks:
                    pend_v.append(issue_xv(c // 3 + 1))
                off = (c % 3) * CH
                xqt = xt_s[:, :, off : off + CH]
                xkt = xt_s[:, :, SUP + off : SUP + off + CH]

                # ---- G = M @ XqT + c1 (feature-major)
                gt = qktp.tile([128, 4, CH], bf16, tag="gt")
                for ec in range(4):
                    psq = ps_proj.tile([128, CH], f32, tag="proj")
                    for dc in range(4):
                        nc.tensor.matmul(
                            psq,
                            mt(dc, ec * 128, (ec + 1) * 128),
                            xqt[:, dc, :],
                            start=(dc == 0),
                            stop=(dc == 3),
                        )
                    nc.scalar.activation(
                        gt[:, ec, :], psq, AF.Identity,
                        bias=c1(ec),
                    )

                # ---- attention per pair: ST -> ET -> Zc, U
                pszc = ps_zc.tile([128, NT], f32, tag="zc")
                u = attnp.tile([128, 4, CH], bf16, tag="u")
                etw = attnp.tile([PAIR, CH], bf16, tag="etw")
                for j in range(NP):
                    ls = slice(j * PAIR, (j + 1) * PAIR)
                    pss = ps_s.tile([PAIR, PAIR], f32, tag="s")
                    for ec in range(4):
                        nc.tensor.matmul(
                            pss,
                            xkt[:, ec, ls],
                            gt[:, ec, ls],
                            start=(ec == 0),
                            stop=False,
                        )
                    nc.scalar.activation(etw[:, ls], pss, AF.Exp, scale=SCALE)
                    nc.vector.tensor_mul(etw[:, ls], etw[:, ls], mask01t)
                    # U[d, l] = sum_m Xv[m, d] * ET[m, l]
                    psu = ps_u.tile([128, 4, PAIR], f32, tag="u")
                    for dc in range(4):
                        nc.tensor.matmul(
                            psu[:, dc, :],
                            xv_s[:, (c % 3) * NP + j, dc * 128 : (dc + 1) * 128],
                            etw[:, ls],
                            start=True,
                            stop=True,
                        )
                    nc.vector.tensor_copy(u[:, :, ls], psu)

                # ---- Zc[token,1] per 128-token tile: ET-stationary @ ones
                # gives the transposed column sums directly (free size 1 on PE)
                for lt in range(NT):
                    nc.tensor.matmul(
                        pszc[:, lt : lt + 1],
                        etw[:, lt * 128 : (lt + 1) * 128],
                        onesc,
                        start=True,
                        stop=True,
                    )
                zcs = attnp.tile([128, NT], f32, tag="zcs")
                nc.vector.reciprocal(zcs, pszc)

                # ---- Y = (U.T @ WVO.T) * (1/Zc) + bob  (token-major, bf16)
                # fused scale+bias on DVE keeps Activation free for gt copies
                for lt in range(NT):
                    lsl = slice(lt * 128, (lt + 1) * 128)
                    psy = ps_y.tile([128, D], f32, tag="y")
                    for dc in range(4):
                        nc.tensor.matmul(
                            psy,
                            u[:, dc, lsl],
                            wvot(dc),
                            start=(dc == 0),
                            stop=(dc == 3),
                        )
                    nc.vector.scalar_tensor_tensor(
                        ysb_s[:, (c % 3) * NT + lt, :], psy, zcs[:, lt : lt + 1],
                        bob,
                        op0=mybir.AluOpType.mult, op1=mybir.AluOpType.add,
                    )
                last_super = c >= nchunks - 3
                if c == nchunks - 1:
                    # final chunk: store each tile as soon as ready, on the
                    # Act HWDGE queue (faster issue than Pool SWDGE gen)
                    for lt2 in range(NT):
                        nc.scalar.dma_start(
                            y_d[t0 + lt2 * 128 : t0 + (lt2 + 1) * 128, :].rearrange(
                                "(t p) e -> p t e", p=128
                            ),
                            ysb_s[:, (c % 3) * NT + lt2 : (c % 3) * NT + lt2 + 1, :],
                        )
                elif last_super:
                    # per-chunk stores in the final super shorten the drain
                    nc.gpsimd.dma_start(
                        y_d[t0 : t0 + CH, :].rearrange("(t p) e -> p t e", p=128),
                        ysb_s[:, (c % 3) * NT : (c % 3 + 1) * NT, :],
                    )
                elif c % 3 == 2:
                    # one SWDGE store per super-chunk on Pool
                    nc.gpsimd.dma_start(
                        y_d[t0 + CH - SUP : t0 + CH, :].rearrange(
                            "(t p) e -> p t e", p=128
                        ),
                        ysb_s,
                    )

    nc.compile()
    return nc


def _host_inputs(queries, keys, values, Wq, bq, Wkv, bkv, Wo, bo, tokens):
    import ml_dtypes

    bf16 = ml_dtypes.bfloat16
    M = Wkv.astype(np.float64).T @ Wq.astype(np.float64)
    WVO = Wo.astype(np.float64) @ Wkv.astype(np.float64)
    # packed const block [128, 4D+4D+D+PAIR+1+4] (see _build)
    CW = 9 * D + PAIR + 5
    CA = 4 * D + PAIR + 5
    cst = np.zeros((128, CW), np.float32)
    cst[:, 0 : 4 * D] = M.T.reshape(4, 128, D).transpose(1, 0, 2).reshape(128, 4 * D)
    m01 = np.ones((PAIR, PAIR), np.float32)
    m01[:L, :L] -= np.eye(L)
    m01[L:, L:] -= np.eye(L)
    m01[:L, L:] = 0.0
    m01[L:, :L] = 0.0
    cst[:PAIR, 4 * D : 4 * D + PAIR] = m01
    cst[:, 4 * D + PAIR] = 1.0
    c1v = Wkv.astype(np.float64).T @ bq.astype(np.float64)
    cst[:, 4 * D + PAIR + 1 : CA] = c1v.reshape(4, 128).T
    cst[:, CA : CA + 4 * D] = (
        WVO.T.reshape(4, 128, D).transpose(1, 0, 2).reshape(128, 4 * D)
    )
    bo_eff = (bo.astype(np.float64) + Wo.astype(np.float64) @ bkv.astype(np.float64))
    cst[:, CA + 4 * D :] = bo_eff[None, :]
    shared = dict(cst=cst.astype(bf16))
    in_maps = []
    for core in range(NCORES):
        m = dict(shared)
        xq = queries[core].reshape(-1, D)[:tokens]
        xk = keys[core].reshape(-1, D)[:tokens]
        nsup = tokens // SUPH
        xqk = np.stack(
            [xq.reshape(nsup, SUPH, D), xk.reshape(nsup, SUPH, D)], axis=1
        ).reshape(2 * tokens, D)
        m["xqk"] = xqk.astype(bf16)
        m["xv"] = values[core].reshape(-1, D)[:tokens].astype(bf16)
        in_maps.append(m)
    return in_maps


def kernel(queries, keys, values, Wq, bq, Wkv, bkv, Wo, bo, _tokens=T, _trace=False):
    queries = np.asarray(queries)
    keys = np.asarray(keys)
    values = np.asarray(values)
    from concourse.bass_utils import run_bass_kernel_spmd

    key = _tokens
    if key not in _CACHE:
        _CACHE[key] = _build(_tokens)
    nc = _CACHE[key]

    in_maps = _host_inputs(
        queries, keys, values,
        np.asarray(Wq), np.asarray(bq), np.asarray(Wkv), np.asarray(bkv),
        np.asarray(Wo), np.asarray(bo), _tokens,
    )
    res = run_bass_kernel_spmd(
        nc, in_maps, core_ids=list(range(NCORES)), trace=_trace,
    )
    outs = [np.asarray(res.results[i]["y"]).astype(np.float32) for i in range(NCORES)]
    if _tokens == T:
        full = np.stack([o.reshape(V, P, L, D) for o in outs], axis=0)
    else:
        full = np.stack(outs, axis=0)
    if _trace:
        return full, res
    return full
